# revision 1
# baseline (speedup 1.0000x reference)
"""Trainium2 Bass kernel for nn_DWNBlock (LRU scan + Lipschitz sandwich MLP).

Self-contained: host-side parameter folding (Cayley transforms, scan
constants) in numpy float64, then a fully-unrolled Tile/Bass SPMD program
on 8 NeuronCores, data-parallel over the batch dimension (one batch
element per core).

Device algorithm per core (x^T in channel-major [128, 8192]):
  1. Bu^(t-major) via PE matmuls with x^T slices as the stationary operand
  2. chunked linear-recurrence scan (L=512): pre-scale by lam^-s (DVE),
     shared upper-triangular-ones matmul over in-chunk time (PE),
     sequential cross-chunk carry (tiny DVE column ops), post-scale by
     lam^t with the carry folded in via per-partition-scalar ops (DVE)
  3. y^T = C_re@H_re - C_im@H_im + D@x^T (PE, PSUM-accumulated)
  4. folded MLP: relu(G1) -> relu(G2) -> relu(G3) -> G4, + residual
All matmuls run in float32r (TF32-class, full PE rate).
"""
import math
import os
import sys

for _p in ('/opt/trn_rl_repo',):
    if _p not in sys.path:
        sys.path.insert(0, _p)

import numpy as np

D = 128          # d_model
N = 128          # d_state
H = 512          # MLP hidden
T = 8192         # sequence length
B = 8            # batch
L = 512          # scan chunk length
NCORES = 8
SCALE = 1.0
SQRT2 = math.sqrt(2.0)

# schedule-tuning knobs (pool buffer counts)
TUNE = dict(epool=2, upool=4, hpool=3, tpool=2, ypool=3, zpool=2, opool=3,
            bups=2, mps=3, mlpps=3)


# ---------------------------------------------------------------- host prep

def _cayley64(W):
    cout, cin = W.shape
    if cin > cout:
        return _cayley64(W.T).T
    U, V = W[:cin], W[cin:]
    I = np.eye(cin, dtype=W.dtype)
    A = U - U.T + V.T @ V
    iIpA = np.linalg.inv(I + A)
    return np.concatenate([iIpA @ (I - A), -2.0 * V @ iIpA], axis=0)


def _host_prep(p):
    """Fold all parameters into device constants (float64 -> float32)."""
    f8 = np.float64
    nu_log = p['nu_log'].astype(f8)
    theta_log = p['theta_log'].astype(f8)
    gamma_log = p['gamma_log'].astype(f8)
    lam = np.exp(-np.exp(nu_log)) * np.exp(1j * np.exp(theta_log))   # [N]
    Beff = np.exp(gamma_log)[:, None] * (p['B_re'].astype(f8) + 1j * p['B_im'].astype(f8))
    beff_w = np.concatenate([Beff.real.T, Beff.imag.T], axis=1)      # [D, 2N]

    C = p['C_re'].astype(f8) + 1j * p['C_im'].astype(f8)             # [D, N]
    ytw = np.concatenate([C.real.T, (-C.imag).T, p['Dmat'].astype(f8).T], axis=1)  # [128, 384]

    s = np.arange(L)
    loglam = np.log(lam)
    pneg = np.exp(-s[:, None] * loglam[None, :])    # [L, N] = lam^-s
    ppos = np.exp(s[None, :] * loglam[:, None])     # [N, L] = lam^t'
    lamL = lam ** L
    lamL1 = lam ** (L - 1)

    # pneg packed for the wide pre-scale over [t-tile pair, re|im|re|im]:
    # pair q covers s-tiles (2q, 2q+1) of the chunk (q in {0,1}).
    def _pk(j):   # s-tile j of pneg, [128, N]
        return pneg[j * 128:(j + 1) * 128, :]
    pneg_pack = np.concatenate([
        # variant A: s-tiles 0,1     [P0r|P0i|P1r|P1i]
        _pk(0).real, _pk(0).imag, _pk(1).real, _pk(1).imag,
        # variant A swapped          [P0i|P0r|P1i|P1r]
        _pk(0).imag, _pk(0).real, _pk(1).imag, _pk(1).real,
        # variant B: s-tiles 2,3
        _pk(2).real, _pk(2).imag, _pk(3).real, _pk(3).imag,
        # variant B swapped
        _pk(2).imag, _pk(2).real, _pk(3).imag, _pk(3).real,
    ], axis=1)                                       # [128, 4096]

    ppos_pack = np.concatenate([ppos.real, ppos.imag, -ppos.imag], axis=1)  # [128, 1536]

    # tri_ones[s, u] = 1 if s <= u, over [128, 512]; block j of the in-chunk
    # triangular matmul uses tri_ones[:, 0:512-128*j] against psum[:, 128*j:512].
    tri_ones = (np.arange(128)[:, None] <= np.arange(512)[None, :]).astype(f8)

    # carry-chain per-partition scalar columns
    lamcols = np.stack([
        lam.real, lam.imag, -lam.imag,
        lamL.real, lamL.imag, -lamL.imag,
        lamL1.real, lamL1.imag, -lamL1.imag,
    ], axis=1)                                       # [128, 9]

    def _q(Wkey, akey, fout):
        Wd = p[Wkey].astype(f8)
        Q = _cayley64((float(p[akey][0]) / np.linalg.norm(Wd)) * Wd)
        return Q[:, fout:], Q[:, :fout]

    Q1in, Q1out = _q('W1', 'alpha1', H)
    Q2in, Q2out = _q('W2', 'alpha2', H)
    Q3in, Q3out = _q('W3', 'alpha3', H)
    Qlin = _cayley64((float(p['alphal'][0]) / np.linalg.norm(p['Wl'].astype(f8)))
                     * p['Wl'].astype(f8))[:, D:]    # [128, 512]

    e = np.exp
    ps1, ps2, ps3 = p['psi1'].astype(f8), p['psi2'].astype(f8), p['psi3'].astype(f8)
    G1 = SCALE * SCALE * SQRT2 * (Q1in.T * e(-ps1)[None, :])                    # [128, 512]
    G2 = 2.0 * SCALE * (e(ps1)[:, None] * Q1out) @ (Q2in.T * e(-ps2)[None, :])  # [512, 512]
    G3 = 2.0 * SCALE * (e(ps2)[:, None] * Q2out) @ (Q3in.T * e(-ps3)[None, :])  # [512, 512]
    G4 = SQRT2 * SCALE * (e(ps3)[:, None] * Q3out) @ Qlin.T                     # [512, 128]

    out = dict(beff_w=beff_w, ytw=ytw, pneg_pack=pneg_pack, ppos_pack=ppos_pack,
               tri_ones=tri_ones, lamcols=lamcols, g1=G1, g2=G2, g3=G3, g4=G4)
    return {k: np.ascontiguousarray(v, dtype=np.float32) for k, v in out.items()}


# ---------------------------------------------------------------- device program

def _build_program(t_len, reps=1):
    from contextlib import nullcontext
    from concourse import bacc
    import concourse.mybir as mybir
    from concourse.tile import TileContext

    f32 = mybir.dt.float32
    f32r = mybir.dt.float32r
    AL = mybir.AluOpType
    ACT = mybir.ActivationFunctionType
    nchunk = t_len // L

    nc = bacc.Bacc("TRN2", target_bir_lowering=False, debug=False)

    xt_d = nc.dram_tensor("xt", [128, t_len], f32r, kind="ExternalInput").ap()
    beff_d = nc.dram_tensor("beff_w", [128, 256], f32r, kind="ExternalInput").ap()
    ytw_d = nc.dram_tensor("ytw", [128, 384], f32r, kind="ExternalInput").ap()
    pneg_d = nc.dram_tensor("pneg_pack", [128, 2048], f32, kind="ExternalInput").ap()
    ppos_d = nc.dram_tensor("ppos_pack", [128, 1536], f32, kind="ExternalInput").ap()
    tri_d = nc.dram_tensor("tri_ones", [128, 512], f32r, kind="ExternalInput").ap()
    lamc_d = nc.dram_tensor("lamcols", [128, 9], f32, kind="ExternalInput").ap()
    g1_d = nc.dram_tensor("g1", [128, 512], f32r, kind="ExternalInput").ap()
    g2_d = nc.dram_tensor("g2", [512, 512], f32r, kind="ExternalInput").ap()
    g3_d = nc.dram_tensor("g3", [512, 512], f32r, kind="ExternalInput").ap()
    g4_d = nc.dram_tensor("g4", [512, 128], f32r, kind="ExternalInput").ap()
    out_d = nc.dram_tensor("outT", [128, t_len], f32, kind="ExternalOutput").ap()

    def r(ap):
        return ap.bitcast(f32r)

    with TileContext(nc) as tc:
        with (
            tc.tile_pool(name="const", bufs=1) as cpool,
            tc.tile_pool(name="epool", bufs=TUNE["epool"]) as epool,
            tc.tile_pool(name="upool", bufs=TUNE["upool"]) as upool,
            tc.tile_pool(name="hpool", bufs=TUNE["hpool"]) as hpool,
            tc.tile_pool(name="tpool", bufs=TUNE["tpool"]) as tpool,
            tc.tile_pool(name="ypool", bufs=TUNE["ypool"]) as ypool,
            tc.tile_pool(name="zpool", bufs=TUNE["zpool"]) as zpool,
            tc.tile_pool(name="opool", bufs=TUNE["opool"]) as opool,
            tc.tile_pool(name="carry", bufs=3) as carry_pool,
            tc.tile_pool(name="bups", bufs=TUNE["bups"], space="PSUM") as bu_ps,
            tc.tile_pool(name="mps", bufs=TUNE["mps"], space="PSUM") as m_ps,
            tc.tile_pool(name="mlpps", bufs=TUNE["mlpps"], space="PSUM") as mlp_ps,
        ):
            # ---- constants into SBUF
            xt = cpool.tile([128, t_len], f32r, tag="xt")
            for q in range(max(1, t_len // 2048)):
                w = min(2048, t_len)
                nc.sync.dma_start(xt[:, q * w:(q + 1) * w], xt_d[:, q * w:(q + 1) * w])
            beff = cpool.tile([128, 256], f32r, tag="beff")
            nc.sync.dma_start(beff[:], beff_d[:])
            ytw = cpool.tile([128, 384], f32r, tag="ytw")
            nc.sync.dma_start(ytw[:], ytw_d[:])
            pneg = cpool.tile([128, 2048], f32, tag="pneg")
            nc.sync.dma_start(pneg[:], pneg_d[:])
            ppos = cpool.tile([128, 1536], f32, tag="ppos")
            nc.sync.dma_start(ppos[:], ppos_d[:])
            tri = cpool.tile([128, 512], f32r, tag="tri")
            nc.sync.dma_start(tri[:], tri_d[:])
            lamc = cpool.tile([128, 9], f32, tag="lamc")
            nc.sync.dma_start(lamc[:], lamc_d[:])
            g1 = cpool.tile([128, 512], f32r, tag="g1")
            nc.sync.dma_start(g1[:], g1_d[:])
            g2 = cpool.tile([128, 2048], f32r, tag="g2")
            for k in range(4):
                nc.sync.dma_start(g2[:, k * 512:(k + 1) * 512], g2_d[k * 128:(k + 1) * 128, :])
            g3 = cpool.tile([128, 2048], f32r, tag="g3")
            for k in range(4):
                nc.sync.dma_start(g3[:, k * 512:(k + 1) * 512], g3_d[k * 128:(k + 1) * 128, :])
            g4 = cpool.tile([128, 512], f32r, tag="g4")
            for k in range(4):
                nc.sync.dma_start(g4[:, k * 128:(k + 1) * 128], g4_d[k * 128:(k + 1) * 128, :])
            bias0 = cpool.tile([128, 1], f32, tag="bias0")
            nc.vector.memset(bias0[:], 0.0)

            # lamcols views
            lam_re, lam_im, lam_imn = lamc[:, 0:1], lamc[:, 1:2], lamc[:, 2:3]
            lamL_re, lamL_im, lamL_imn = lamc[:, 3:4], lamc[:, 4:5], lamc[:, 5:6]
            lamL1_re, lamL1_im, lamL1_imn = lamc[:, 6:7], lamc[:, 7:8], lamc[:, 8:9]

            ppos_re = ppos[:, 0:512]
            ppos_im = ppos[:, 512:1024]
            ppos_imn = ppos[:, 1024:1536]

            loop_cm = tc.For_i(0, reps) if reps > 1 else nullcontext()
            with loop_cm:
                # carry state h_0 = 0
                h_re = carry_pool.tile([128, 1], f32, tag="hre")
                h_im = carry_pool.tile([128, 1], f32, tag="him")
                nc.vector.memset(h_re[:], 0.0)
                nc.vector.memset(h_im[:], 0.0)
                _chunk_loop_body(nc, tc, locals())

    nc.finalize()
    return nc


def _chunk_loop_body(nc, tc, env):
    """The per-core pipeline over chunks (split out so a timing build can
    wrap it in a repeat loop)."""
    import concourse.mybir as mybir
    f32 = mybir.dt.float32
    f32r = mybir.dt.float32r
    AL = mybir.AluOpType
    ACT = mybir.ActivationFunctionType
    (nchunk, xt, beff, ytw, pneg, ppos, tri, lamc, g1, g2, g3, g4, bias0, out_d,
     epool, upool, hpool, tpool, ypool, zpool, opool, carry_pool,
     bu_ps, m_ps, mlp_ps, h_re, h_im,
     lam_re, lam_im, lam_imn, lamL_re, lamL_im, lamL_imn,
     lamL1_re, lamL1_im, lamL1_imn, ppos_re, ppos_im, ppos_imn) = (
        env['nchunk'], env['xt'], env['beff'], env['ytw'], env['pneg'],
        env['ppos'], env['tri'], env['lamc'], env['g1'], env['g2'], env['g3'],
        env['g4'], env['bias0'], env['out_d'],
        env['epool'], env['upool'], env['hpool'], env['tpool'], env['ypool'],
        env['zpool'], env['opool'], env['carry_pool'],
        env['bu_ps'], env['m_ps'], env['mlp_ps'], env['h_re'], env['h_im'],
        env['lam_re'], env['lam_im'], env['lam_imn'], env['lamL_re'],
        env['lamL_im'], env['lamL_imn'], env['lamL1_re'], env['lamL1_im'],
        env['lamL1_imn'], env['ppos_re'], env['ppos_im'], env['ppos_imn'])
    L = 512

    def r(ap):
        return ap if ap.dtype == f32r else ap.bitcast(f32r)

    if True:
            for c in range(nchunk):
                t0 = c * L
                # ---- Bu + pre-scale: 2 pairs of t-tiles
                upair = []
                for q in range(2):
                    bu = bu_ps.tile([128, 512], f32, tag="bu")
                    for half in range(2):
                        i = 2 * q + half          # s-tile index in chunk
                        lhs = xt[:, t0 + i * 128: t0 + (i + 1) * 128]
                        nc.tensor.matmul(bu[:, half * 256:(half + 1) * 256],
                                         r(lhs), r(beff[:]), start=True, stop=True)
                    e1 = epool.tile([128, 512], f32, tag="e1")
                    e2 = epool.tile([128, 512], f32, tag="e2")
                    pv = pneg[:, q * 1024: q * 1024 + 512]
                    pv_sw = pneg[:, q * 1024 + 512: q * 1024 + 1024]
                    nc.vector.tensor_tensor(e1[:], bu[:], pv, AL.mult)
                    nc.vector.tensor_tensor(e2[:], bu[:], pv_sw, AL.mult)
                    up = upool.tile([128, 512], f32r, tag="upair")
                    # re parts (DVE), im parts (GPSIMD, SBUF-only)
                    for half in range(2):
                        o = half * 256
                        nc.vector.tensor_tensor(up[:, o:o + 128], e1[:, o:o + 128],
                                                e1[:, o + 128:o + 256], AL.subtract)
                        nc.gpsimd.tensor_tensor(up[:, o + 128:o + 256], e2[:, o:o + 128],
                                                e2[:, o + 128:o + 256], AL.add)
                    upair.append(up)

                # ---- in-chunk triangular scan matmuls
                m_re = m_ps.tile([128, 512], f32, tag="m")
                m_im = m_ps.tile([128, 512], f32, tag="m")
                for j in range(4):
                    up = upair[j // 2]
                    o = (j % 2) * 256
                    width = 512 - 128 * j
                    nc.tensor.matmul(m_re[:, 128 * j:512], r(up[:, o:o + 128]),
                                     r(tri[:, 0:width]), start=(j == 0), stop=(j == 3))
                for j in range(4):
                    up = upair[j // 2]
                    o = (j % 2) * 256 + 128
                    width = 512 - 128 * j
                    nc.tensor.matmul(m_im[:, 128 * j:512], r(up[:, o:o + 128]),
                                     r(tri[:, 0:width]), start=(j == 0), stop=(j == 3))

                # ---- carry fold values a = lam * h   (tiny column ops)
                a_re = carry_pool.tile([128, 1], f32, tag="are")
                a_im = carry_pool.tile([128, 1], f32, tag="aim")
                tmp1 = carry_pool.tile([128, 1], f32, tag="ctmp1")
                tmp2 = carry_pool.tile([128, 1], f32, tag="ctmp2")
                nc.vector.tensor_tensor(tmp1[:], h_re[:], lam_re, AL.mult)
                nc.vector.scalar_tensor_tensor(a_re[:], h_im[:], lam_imn, tmp1[:],
                                               AL.mult, AL.add)
                nc.vector.tensor_tensor(tmp2[:], h_re[:], lam_im, AL.mult)
                nc.vector.scalar_tensor_tensor(a_im[:], h_im[:], lam_re, tmp2[:],
                                               AL.mult, AL.add)

                # ---- next carry h' = lamL*h + lamL1*M[:, L-1]
                # m-dependent half computed independently of h (starts as soon
                # as the scan matmuls land), so the serial h->h' depth is 3.
                mr_col = m_re[:, L - 1:L]
                mi_col = m_im[:, L - 1:L]
                mp1 = carry_pool.tile([128, 1], f32, tag="mp1")
                mp_re = carry_pool.tile([128, 1], f32, tag="mpre")
                mp2 = carry_pool.tile([128, 1], f32, tag="mp2")
                mp_im = carry_pool.tile([128, 1], f32, tag="mpim")
                nc.vector.tensor_tensor(mp1[:], mr_col, lamL1_re, AL.mult)
                nc.vector.scalar_tensor_tensor(mp_re[:], mi_col, lamL1_imn, mp1[:], AL.mult, AL.add)
                nc.vector.tensor_tensor(mp2[:], mi_col, lamL1_re, AL.mult)
                nc.vector.scalar_tensor_tensor(mp_im[:], mr_col, lamL1_im, mp2[:], AL.mult, AL.add)
                c1 = carry_pool.tile([128, 1], f32, tag="c1")
                h_re_n = carry_pool.tile([128, 1], f32, tag="hre")
                d1 = carry_pool.tile([128, 1], f32, tag="d1")
                h_im_n = carry_pool.tile([128, 1], f32, tag="him")
                nc.vector.scalar_tensor_tensor(c1[:], h_re[:], lamL_re, mp_re[:], AL.mult, AL.add)
                nc.vector.scalar_tensor_tensor(h_re_n[:], h_im[:], lamL_imn, c1[:], AL.mult, AL.add)
                nc.vector.scalar_tensor_tensor(d1[:], h_im[:], lamL_re, mp_im[:], AL.mult, AL.add)
                nc.vector.scalar_tensor_tensor(h_im_n[:], h_re[:], lamL_im, d1[:], AL.mult, AL.add)

                # ---- post-scale: H = ppos * (M + bcast(a))
                t1 = tpool.tile([128, 512], f32, tag="t1")
                t2 = tpool.tile([128, 512], f32, tag="t2")
                t3 = tpool.tile([128, 512], f32, tag="t3")
                t4 = tpool.tile([128, 512], f32, tag="t4")
                hre_t = hpool.tile([128, 512], f32r, tag="Hre")
                him_t = hpool.tile([128, 512], f32r, tag="Him")
                nc.vector.scalar_tensor_tensor(t1[:], m_re[:], a_re[:], ppos_re, AL.add, AL.mult)
                nc.vector.scalar_tensor_tensor(t2[:], m_im[:], a_im[:], ppos_imn, AL.add, AL.mult)
                nc.gpsimd.tensor_tensor(hre_t[:], t1[:], t2[:], AL.add)
                nc.vector.scalar_tensor_tensor(t3[:], m_re[:], a_re[:], ppos_im, AL.add, AL.mult)
                nc.vector.scalar_tensor_tensor(t4[:], m_im[:], a_im[:], ppos_re, AL.add, AL.mult)
                nc.gpsimd.tensor_tensor(him_t[:], t3[:], t4[:], AL.add)

                h_re, h_im = h_re_n, h_im_n

                # ---- y^T = Cre@Hre - Cim@Him + D@xT
                y_ps = mlp_ps.tile([128, 512], f32, tag="mlp")
                nc.tensor.matmul(y_ps[:], r(ytw[:, 0:128]), r(hre_t[:]), start=True, stop=False)
                nc.tensor.matmul(y_ps[:], r(ytw[:, 128:256]), r(him_t[:]), start=False, stop=False)
                nc.tensor.matmul(y_ps[:], r(ytw[:, 256:384]), r(xt[:, t0:t0 + 512]),
                                 start=False, stop=True)
                y_sb = ypool.tile([128, 512], f32r, tag="ysb")
                nc.scalar.copy(y_sb[:], y_ps[:])

                # ---- MLP
                z1 = []
                for m in range(4):
                    zp = mlp_ps.tile([128, 512], f32, tag="mlp")
                    nc.tensor.matmul(zp[:], r(g1[:, m * 128:(m + 1) * 128]), r(y_sb[:]),
                                     start=True, stop=True)
                    zt = zpool.tile([128, 512], f32r, tag=f"z1_{m}")
                    nc.scalar.activation(zt[:], zp[:], ACT.Relu, bias=bias0[:])
                    z1.append(zt)
                z2 = []
                for m in range(4):
                    zp = mlp_ps.tile([128, 512], f32, tag="mlp")
                    for k in range(4):
                        nc.tensor.matmul(zp[:], r(g2[:, k * 512 + m * 128: k * 512 + (m + 1) * 128]),
                                         r(z1[k][:]), start=(k == 0), stop=(k == 3))
                    zt = zpool.tile([128, 512], f32r, tag=f"z2_{m}")
                    nc.scalar.activation(zt[:], zp[:], ACT.Relu, bias=bias0[:])
                    z2.append(zt)
                z3 = []
                for m in range(4):
                    zp = mlp_ps.tile([128, 512], f32, tag="mlp")
                    for k in range(4):
                        nc.tensor.matmul(zp[:], r(g3[:, k * 512 + m * 128: k * 512 + (m + 1) * 128]),
                                         r(z2[k][:]), start=(k == 0), stop=(k == 3))
                    zt = zpool.tile([128, 512], f32r, tag=f"z3_{m}")
                    nc.scalar.activation(zt[:], zp[:], ACT.Relu, bias=bias0[:])
                    z3.append(zt)
                zp = mlp_ps.tile([128, 512], f32, tag="mlp")
                for k in range(4):
                    nc.tensor.matmul(zp[:], r(g4[:, k * 128:(k + 1) * 128]), r(z3[k][:]),
                                     start=(k == 0), stop=(k == 3))
                o_sb = opool.tile([128, 512], f32, tag="osb")
                nc.vector.tensor_tensor(o_sb[:], zp[:], xt[:, t0:t0 + 512].bitcast(f32), AL.add)
                nc.sync.dma_start(out_d[:, t0:t0 + 512], o_sb[:])



# ---------------------------------------------------------------- PJRT runner

def _make_runner(nc, n_cores):
    import jax
    from jax.sharding import Mesh, PartitionSpec
    from jax.experimental.shard_map import shard_map
    import concourse.mybir as mybir
    from concourse import bass2jax

    bass2jax.install_neuronx_cc_hook()
    assert nc.is_finalized()
    partition_name = nc.partition_id_tensor.name if nc.partition_id_tensor else None

    in_names, out_names, out_avals, zero_shapes = [], [], [], []
    for alloc in nc.m.functions[0].allocations:
        if not isinstance(alloc, mybir.MemoryLocationSet):
            continue
        name = alloc.memorylocations[0].name
        if alloc.kind == "ExternalInput":
            if name != partition_name:
                in_names.append(name)
        elif alloc.kind == "ExternalOutput":
            shape = tuple(alloc.tensor_shape)
            dtype = mybir.dt.np(alloc.dtype)
            out_names.append(name)
            out_avals.append(jax.core.ShapedArray(shape, dtype))
            zero_shapes.append((shape, dtype))
    n_params = len(in_names)
    n_outs = len(out_avals)
    all_in_names = list(in_names) + list(out_names)
    if partition_name is not None:
        all_in_names.append(partition_name)
    donate = tuple(range(n_params, n_params + n_outs))

    def _body(*args):
        operands = list(args)
        if partition_name is not None:
            operands.append(bass2jax.partition_id_tensor())
        outs = bass2jax._bass_exec_p.bind(
            *operands,
            out_avals=tuple(out_avals),
            in_names=tuple(all_in_names),
            out_names=tuple(out_names),
            lowering_input_output_aliases=(),
            sim_require_finite=True,
            sim_require_nnan=True,
            nc=nc,
        )
        return tuple(outs)

    devices = jax.devices()[:n_cores]
    if n_cores == 1:
        fn = jax.jit(_body, donate_argnums=donate, keep_unused=True)
    else:
        mesh = Mesh(np.asarray(devices), ("core",))
        fn = jax.jit(
            shard_map(_body, mesh=mesh,
                      in_specs=(PartitionSpec("core"),) * (n_params + n_outs),
                      out_specs=(PartitionSpec("core"),) * n_outs,
                      check_rep=False),
            donate_argnums=donate, keep_unused=True,
        )

    def run(per_core_inputs):
        if n_cores == 1:
            ins = [np.asarray(per_core_inputs[0][n]) for n in in_names]
            zeros = [np.zeros(s, d) for s, d in zero_shapes]
        else:
            ins = [np.concatenate([np.asarray(per_core_inputs[c][n])
                                   for c in range(n_cores)], axis=0) for n in in_names]
            zeros = [np.zeros((n_cores * s[0], *s[1:]), d) for s, d in zero_shapes]
        out_arrs = fn(*ins, *zeros)
        if n_cores == 1:
            return [{name: np.asarray(out_arrs[i]) for i, name in enumerate(out_names)}]
        res = []
        for c in range(n_cores):
            d = {}
            for i, name in enumerate(out_names):
                full = np.asarray(out_arrs[i])
                d[name] = full.reshape(n_cores, *out_avals[i].shape)[c]
            res.append(d)
        return res

    run.fn = fn
    run.in_names = in_names
    run.out_names = out_names
    run.zero_shapes = zero_shapes
    return run


_RUNNER = None


def _get_runner():
    global _RUNNER
    if _RUNNER is None:
        nc = _build_program(T)
        _RUNNER = _make_runner(nc, NCORES)
    return _RUNNER


def kernel(**inputs):
    import time as _time
    global _RUNNER
    p = {k: np.asarray(v) for k, v in inputs.items()}
    consts = _host_prep(p)
    x = p['x'].astype(np.float32)            # [B, T, D]
    per_core = []
    for b in range(B):
        m = dict(consts)
        m['xt'] = np.ascontiguousarray(x[b].T)
        per_core.append(m)
    res = None
    for attempt in range(3):
        try:
            run = _get_runner()
            res = run(per_core)
            break
        except Exception:
            # transient NRT exec faults have been observed on the first
            # execution of a freshly compiled NEFF; rebuild the jitted
            # callable (NEFF comes from the compile cache) and retry.
            _RUNNER = None
            if attempt == 2:
                raise
            _time.sleep(2.0)
    out = np.stack([res[b]['outT'].T for b in range(B)], axis=0)
    return np.ascontiguousarray(out, dtype=np.float32)



# revision 9
# speedup vs baseline: 1.1029x; 1.1029x over previous
"""Trainium2 Bass kernel for nn_DWNBlock (LRU scan + Lipschitz sandwich MLP).

Self-contained: host-side parameter folding (Cayley transforms, scan
constants) in numpy float64, then a fully-unrolled Tile/Bass SPMD program
on 8 NeuronCores, data-parallel over the batch dimension (one batch
element per core).

Device algorithm per core (x^T in channel-major [128, 8192]):
  1. Bu^(t-major) via PE matmuls (f32r), pre-scale by lam^-s (DVE, one
     merged broadcast op), complex combine into bf16 `up` (Pool)
  2. chunked linear-recurrence scan (L=512): shared upper-triangular-ones
     matmuls over in-chunk time (PE, bf16), sequential cross-chunk carry
     (tiny Pool/DVE column ops), post-scale by lam^t with the carry folded
     in (2 merged DVE ops + 1 Pool add)
  3. y^T = C_re@H_re - C_im@H_im + D@x^T (PE f32r, PSUM-accumulated)
  4. folded MLP in fp8e4m3 DoubleRow perf mode (2x PE rate):
     relu(G1 f32r) -> fp8; G2, G3 fp8-DR + relu->fp8; G4 fp8-DR;
     residual add + DMA out
Issue order is software-pipelined two chunks deep so the PE never waits
on the DVE/Pool pre-scale chain: per slot c the PE runs
[bu_c | y,G1_{c-2} | tri_{c-1} | G2..G4_{c-2}].
"""
import math
import os
import sys

for _p in ('/opt/trn_rl_repo',):
    if _p not in sys.path:
        sys.path.insert(0, _p)

import numpy as np
import ml_dtypes

D = 128          # d_model
N = 128          # d_state
H = 512          # MLP hidden
T = 8192         # sequence length
B = 8            # batch
L = 512          # scan chunk length
NCORES = 8
SCALE = 1.0
SQRT2 = math.sqrt(2.0)

E4M3 = ml_dtypes.float8_e4m3
BF16 = ml_dtypes.bfloat16

# schedule-tuning knobs
TUNE = dict(epool=2, upool=2, hpool=3, tpool=3, ypool=3, zpool=3, opool=3,
            bups=1, mps=2, zps=2,
            relu_pair=True,        # [128,1024] paired relus vs 4x[128,512]
            g2_kp_interleave=True)  # all-kp0 mms before kp1 mms in G2/G3


# ---------------------------------------------------------------- host prep

def _cayley64(W):
    cout, cin = W.shape
    if cin > cout:
        return _cayley64(W.T).T
    U, V = W[:cin], W[cin:]
    I = np.eye(cin, dtype=W.dtype)
    A = U - U.T + V.T @ V
    iIpA = np.linalg.inv(I + A)
    return np.concatenate([iIpA @ (I - A), -2.0 * V @ iIpA], axis=0)


def _host_prep(p):
    """Fold all parameters into device constants (float64 -> device dtypes)."""
    f8 = np.float64
    nu_log = p['nu_log'].astype(f8)
    theta_log = p['theta_log'].astype(f8)
    gamma_log = p['gamma_log'].astype(f8)
    lam = np.exp(-np.exp(nu_log)) * np.exp(1j * np.exp(theta_log))   # [N]
    Beff = np.exp(gamma_log)[:, None] * (p['B_re'].astype(f8) + 1j * p['B_im'].astype(f8))
    beff_w = np.concatenate([Beff.real.T, Beff.imag.T], axis=1)      # [D, 2N]

    C = p['C_re'].astype(f8) + 1j * p['C_im'].astype(f8)             # [D, N]
    ytw = np.concatenate([C.real.T, (-C.imag).T, p['Dmat'].astype(f8).T], axis=1)  # [128, 384]

    s = np.arange(L)
    loglam = np.log(lam)
    pneg = np.exp(-s[:, None] * loglam[None, :])    # [L, N] = lam^-s
    ppos = np.exp(s[None, :] * loglam[:, None])     # [N, L] = lam^t'
    lamL = lam ** L
    lamL1 = lam ** (L - 1)

    # pneg packed [128, 2048] = [variant0 (1024) | variant1 (1024)]
    #   variant0: [P0r|P0i|P1r|P1i|P2r|P2i|P3r|P3i]  (for e1 = bu * pneg)
    #   variant1: [P0i|P0r|P1i|P1r|P2i|P2r|P3i|P3r]  (for e2 = bu * pneg_sw)
    def _pk(j):   # s-tile j of pneg, [128, N]
        return pneg[j * 128:(j + 1) * 128, :]
    v0 = np.concatenate([a for j in range(4) for a in (_pk(j).real, _pk(j).imag)], axis=1)
    v1 = np.concatenate([a for j in range(4) for a in (_pk(j).imag, _pk(j).real)], axis=1)
    pneg_pack = np.concatenate([v0, v1], axis=1)                      # [128, 2048]

    # ppos packed [128, 2048] = [re | im | -im | re]
    # t13 = (m_re + a_re) * [re|im] ; t24 = (m_im + a_im) * [-im|re]
    # H = t13 + t24 = [Hre | Him]
    ppos_pack = np.concatenate([ppos.real, ppos.imag, -ppos.imag, ppos.real], axis=1)

    # tri_ones[s, u] = 1 if s <= u, [128, 512] (bf16 on device)
    tri_ones = (np.arange(128)[:, None] <= np.arange(512)[None, :]).astype(f8)

    # carry-chain per-partition scalar column pairs:
    # [lam_re|lam_im], [-lam_im|lam_re], [lamL_re|lamL_im], [-lamL_im|lamL_re],
    # [lamL1_re|lamL1_im], [-lamL1_im|lamL1_re]
    lamcols = np.stack([
        lam.real, lam.imag, -lam.imag, lam.real,
        lamL.real, lamL.imag, -lamL.imag, lamL.real,
        lamL1.real, lamL1.imag, -lamL1.imag, lamL1.real,
    ], axis=1)                                       # [128, 12]

    def _q(Wkey, akey, fout):
        Wd = p[Wkey].astype(f8)
        Q = _cayley64((float(p[akey][0]) / np.linalg.norm(Wd)) * Wd)
        return Q[:, fout:], Q[:, :fout]

    Q1in, Q1out = _q('W1', 'alpha1', H)
    Q2in, Q2out = _q('W2', 'alpha2', H)
    Q3in, Q3out = _q('W3', 'alpha3', H)
    Qlin = _cayley64((float(p['alphal'][0]) / np.linalg.norm(p['Wl'].astype(f8)))
                     * p['Wl'].astype(f8))[:, D:]    # [128, 512]

    e = np.exp
    ps1, ps2, ps3 = p['psi1'].astype(f8), p['psi2'].astype(f8), p['psi3'].astype(f8)
    G1 = SCALE * SCALE * SQRT2 * (Q1in.T * e(-ps1)[None, :])                    # [128, 512]
    G2 = 2.0 * SCALE * (e(ps1)[:, None] * Q1out) @ (Q2in.T * e(-ps2)[None, :])  # [512, 512]
    G3 = 2.0 * SCALE * (e(ps2)[:, None] * Q2out) @ (Q3in.T * e(-ps3)[None, :])  # [512, 512]
    G4 = SQRT2 * SCALE * (e(ps3)[:, None] * Q3out) @ Qlin.T                     # [512, 128]

    def _kpack(G, w):
        # [K, w] (K = 4*128) -> [128, 4*w] with [p, 512k + j] = G[128k + p, j]
        return np.concatenate([G[128 * k:128 * (k + 1), :] for k in range(4)], axis=1)

    out = {}
    for k, v in dict(beff_w=beff_w, ytw=ytw, pneg_pack=pneg_pack,
                     ppos_pack=ppos_pack, lamcols=lamcols, g1=G1).items():
        out[k] = np.ascontiguousarray(v, dtype=np.float32)
    out['tri_ones'] = np.ascontiguousarray(tri_ones, dtype=BF16)
    out['g2_8'] = np.ascontiguousarray(_kpack(G2, 512), dtype=np.float32).astype(E4M3)
    out['g3_8'] = np.ascontiguousarray(_kpack(G3, 512), dtype=np.float32).astype(E4M3)
    out['g4_8'] = np.ascontiguousarray(_kpack(G4, 128), dtype=np.float32).astype(E4M3)
    return out


# ---------------------------------------------------------------- device program

def _build_program(t_len, reps=1):
    from contextlib import nullcontext
    from concourse import bacc
    import concourse.mybir as mybir
    from concourse.tile import TileContext

    f32 = mybir.dt.float32
    f32r = mybir.dt.float32r
    bf16 = mybir.dt.bfloat16
    fp8 = mybir.dt.float8e4
    AL = mybir.AluOpType
    ACT = mybir.ActivationFunctionType
    DR = mybir.MatmulPerfMode.DoubleRow
    nchunk = t_len // L

    nc = bacc.Bacc("TRN2", target_bir_lowering=False, debug=False)

    xt_d = nc.dram_tensor("xt", [128, t_len], f32r, kind="ExternalInput").ap()
    beff_d = nc.dram_tensor("beff_w", [128, 256], f32r, kind="ExternalInput").ap()
    ytw_d = nc.dram_tensor("ytw", [128, 384], f32r, kind="ExternalInput").ap()
    pneg_d = nc.dram_tensor("pneg_pack", [128, 2048], f32, kind="ExternalInput").ap()
    ppos_d = nc.dram_tensor("ppos_pack", [128, 2048], f32, kind="ExternalInput").ap()
    tri_d = nc.dram_tensor("tri_ones", [128, 512], bf16, kind="ExternalInput").ap()
    lamc_d = nc.dram_tensor("lamcols", [128, 12], f32, kind="ExternalInput").ap()
    g1_d = nc.dram_tensor("g1", [128, 512], f32r, kind="ExternalInput").ap()
    g2_d = nc.dram_tensor("g2_8", [128, 2048], fp8, kind="ExternalInput").ap()
    g3_d = nc.dram_tensor("g3_8", [128, 2048], fp8, kind="ExternalInput").ap()
    g4_d = nc.dram_tensor("g4_8", [128, 512], fp8, kind="ExternalInput").ap()
    out_d = nc.dram_tensor("outT", [128, t_len], f32, kind="ExternalOutput").ap()

    with TileContext(nc) as tc:
        with (
            tc.tile_pool(name="const", bufs=1) as cpool,
            tc.tile_pool(name="epool", bufs=TUNE["epool"]) as epool,
            tc.tile_pool(name="upool", bufs=TUNE["upool"]) as upool,
            tc.tile_pool(name="hpool", bufs=TUNE["hpool"]) as hpool,
            tc.tile_pool(name="tpool", bufs=TUNE["tpool"]) as tpool,
            tc.tile_pool(name="ypool", bufs=TUNE["ypool"]) as ypool,
            tc.tile_pool(name="zpool", bufs=TUNE["zpool"]) as zpool,
            tc.tile_pool(name="opool", bufs=TUNE["opool"]) as opool,
            tc.tile_pool(name="carry", bufs=3) as carry_pool,
            tc.tile_pool(name="bups", bufs=TUNE["bups"], space="PSUM") as bu_ps,
            tc.tile_pool(name="mps", bufs=TUNE["mps"], space="PSUM") as m_ps,
            tc.tile_pool(name="zps", bufs=TUNE["zps"], space="PSUM") as z_ps,
        ):
            # ---- constants into SBUF
            xt = cpool.tile([128, t_len], f32r, tag="xt")
            for q in range(max(1, t_len // 2048)):
                w = min(2048, t_len)
                nc.sync.dma_start(xt[:, q * w:(q + 1) * w], xt_d[:, q * w:(q + 1) * w])
            beff = cpool.tile([128, 256], f32r, tag="beff")
            nc.sync.dma_start(beff[:], beff_d[:])
            ytw = cpool.tile([128, 384], f32r, tag="ytw")
            nc.sync.dma_start(ytw[:], ytw_d[:])
            pneg3 = cpool.tile([128, 2, 1024], f32, tag="pneg")
            nc.sync.dma_start(pneg3[:], pneg_d[:])
            ppos3 = cpool.tile([128, 4, 512], f32, tag="ppos")
            nc.sync.dma_start(ppos3[:], ppos_d[:])
            tri = cpool.tile([128, 512], bf16, tag="tri")
            nc.sync.dma_start(tri[:], tri_d[:])
            lamc = cpool.tile([128, 12], f32, tag="lamc")
            nc.sync.dma_start(lamc[:], lamc_d[:])
            g1 = cpool.tile([128, 512], f32r, tag="g1")
            nc.sync.dma_start(g1[:], g1_d[:])
            g2t = cpool.tile([128, 4, 512], fp8, tag="g2")
            nc.sync.dma_start(g2t[:], g2_d[:])
            g3t = cpool.tile([128, 4, 512], fp8, tag="g3")
            nc.sync.dma_start(g3t[:], g3_d[:])
            g4t = cpool.tile([128, 4, 128], fp8, tag="g4")
            nc.sync.dma_start(g4t[:], g4_d[:])
            bias0 = cpool.tile([128, 1], f32, tag="bias0")
            nc.vector.memset(bias0[:], 0.0)

            env = dict(nc=nc, mybir=mybir, nchunk=nchunk, xt=xt, beff=beff,
                       ytw=ytw, pneg3=pneg3, ppos3=ppos3, tri=tri, lamc=lamc,
                       g1=g1, g2t=g2t, g3t=g3t, g4t=g4t, bias0=bias0,
                       out_d=out_d, epool=epool, upool=upool, hpool=hpool,
                       tpool=tpool, ypool=ypool, zpool=zpool, opool=opool,
                       carry_pool=carry_pool, bu_ps=bu_ps, m_ps=m_ps, z_ps=z_ps)

            loop_cm = tc.For_i(0, reps) if reps > 1 else nullcontext()
            with loop_cm:
                _chunk_loop_body(env)

    nc.finalize()
    return nc


def _chunk_loop_body(env):
    nc = env['nc']
    mybir = env['mybir']
    f32 = mybir.dt.float32
    f32r = mybir.dt.float32r
    bf16 = mybir.dt.bfloat16
    fp8 = mybir.dt.float8e4
    AL = mybir.AluOpType
    ACT = mybir.ActivationFunctionType
    DR = mybir.MatmulPerfMode.DoubleRow

    nchunk = env['nchunk']
    xt, beff, ytw = env['xt'], env['beff'], env['ytw']
    pneg3, ppos3, tri, lamc = env['pneg3'], env['ppos3'], env['tri'], env['lamc']
    g1, g2t, g3t, g4t, bias0 = env['g1'], env['g2t'], env['g3t'], env['g4t'], env['bias0']
    out_d = env['out_d']
    epool, upool, hpool, tpool = env['epool'], env['upool'], env['hpool'], env['tpool']
    ypool, zpool, opool, carry_pool = env['ypool'], env['zpool'], env['opool'], env['carry_pool']
    bu_ps, m_ps, z_ps = env['bu_ps'], env['m_ps'], env['z_ps']

    # lamcols column-pair views: [x_re|x_im] and [-x_im|x_re]
    lam_p, lam_q = lamc[:, 0:2], lamc[:, 2:4]
    lamL_p, lamL_q = lamc[:, 4:6], lamc[:, 6:8]
    lamL1_p, lamL1_q = lamc[:, 8:10], lamc[:, 10:12]

    # carry state h_0 = 0   ([128, 2] = [re|im])
    h2 = carry_pool.tile([128, 2], f32, tag="h2")
    nc.vector.memset(h2[:], 0.0)

    def issue_bu(c):
        """Bu matmuls + lam^-s pre-scale + complex combine -> up (bf16)."""
        t0 = c * L
        bu = bu_ps.tile([128, 1024], f32, tag="bu")
        for i in range(4):
            lhs = xt[:, t0 + i * 128: t0 + (i + 1) * 128]
            nc.tensor.matmul(bu[:, i * 256:(i + 1) * 256], lhs, beff[:],
                             start=True, stop=True)
        e12 = epool.tile([128, 2, 1024], f32, tag="e12")
        nc.vector.tensor_tensor(e12[:], bu[:, None, :].broadcast_to([128, 2, 1024]),
                                pneg3[:], AL.mult)
        up = upool.tile([128, 1024], bf16, tag="up")
        for i in range(4):
            o = i * 256
            nc.gpsimd.tensor_tensor(up[:, o:o + 128], e12[:, 0, o:o + 128],
                                    e12[:, 0, o + 128:o + 256], AL.subtract)
            nc.gpsimd.tensor_tensor(up[:, o + 128:o + 256], e12[:, 1, o:o + 128],
                                    e12[:, 1, o + 128:o + 256], AL.add)
        return up

    def issue_scan(c, up, h2):
        """tri matmuls, carry chain, post-scale -> H [128, 2, 512] f32r."""
        m_re = m_ps.tile([128, 512], f32, tag="m")
        m_im = m_ps.tile([128, 512], f32, tag="m")
        for j in range(4):
            o = j * 256
            nc.tensor.matmul(m_re[:, 128 * j:512], up[:, o:o + 128],
                             tri[:, 0:512 - 128 * j], start=(j == 0), stop=(j == 3))
        for j in range(4):
            o = j * 256 + 128
            nc.tensor.matmul(m_im[:, 128 * j:512], up[:, o:o + 128],
                             tri[:, 0:512 - 128 * j], start=(j == 0), stop=(j == 3))

        # a = lam * h  (Pool, SBUF-only; [128,2] complex as [re|im])
        ppa = carry_pool.tile([128, 2], f32, tag="ppa")
        ppb = carry_pool.tile([128, 2], f32, tag="ppb")
        a2 = carry_pool.tile([128, 2], f32, tag="a2")
        nc.gpsimd.tensor_tensor(ppa[:], h2[:, 0:1].broadcast_to([128, 2]), lam_p, AL.mult)
        nc.gpsimd.tensor_tensor(ppb[:], h2[:, 1:2].broadcast_to([128, 2]), lam_q, AL.mult)
        nc.gpsimd.tensor_tensor(a2[:], ppa[:], ppb[:], AL.add)

        # mp = lamL1 * M[:, L-1]   (DVE: reads PSUM column)
        mr_col = m_re[:, L - 1:L]
        mi_col = m_im[:, L - 1:L]
        ppe = carry_pool.tile([128, 2], f32, tag="ppe")
        ppf = carry_pool.tile([128, 2], f32, tag="ppf")
        mp2 = carry_pool.tile([128, 2], f32, tag="mp2")
        nc.vector.tensor_tensor(ppe[:], mr_col.broadcast_to([128, 2]), lamL1_p, AL.mult)
        nc.vector.tensor_tensor(ppf[:], mi_col.broadcast_to([128, 2]), lamL1_q, AL.mult)
        nc.vector.tensor_tensor(mp2[:], ppe[:], ppf[:], AL.add)

        # h' = lamL * h + mp  (Pool)
        ppc = carry_pool.tile([128, 2], f32, tag="ppc")
        ppd = carry_pool.tile([128, 2], f32, tag="ppd")
        s2 = carry_pool.tile([128, 2], f32, tag="s2")
        h2_n = carry_pool.tile([128, 2], f32, tag="h2")
        nc.gpsimd.tensor_tensor(ppc[:], h2[:, 0:1].broadcast_to([128, 2]), lamL_p, AL.mult)
        nc.gpsimd.tensor_tensor(ppd[:], h2[:, 1:2].broadcast_to([128, 2]), lamL_q, AL.mult)
        nc.gpsimd.tensor_tensor(s2[:], ppc[:], ppd[:], AL.add)
        nc.gpsimd.tensor_tensor(h2_n[:], s2[:], mp2[:], AL.add)

        # post-scale: t13 = (m_re + a_re) * [re|im], t24 = (m_im + a_im) * [-im|re]
        t13 = tpool.tile([128, 2, 512], f32, tag="t13")
        t24 = tpool.tile([128, 2, 512], f32, tag="t24")
        nc.vector.scalar_tensor_tensor(t13[:], m_re[:, None, :].broadcast_to([128, 2, 512]),
                                       a2[:, 0:1], ppos3[:, 0:2, :], AL.add, AL.mult)
        nc.vector.scalar_tensor_tensor(t24[:], m_im[:, None, :].broadcast_to([128, 2, 512]),
                                       a2[:, 1:2], ppos3[:, 2:4, :], AL.add, AL.mult)
        Ht = hpool.tile([128, 2, 512], f32r, tag="H")
        nc.gpsimd.tensor_tensor(Ht[:], t13[:], t24[:], AL.add)
        return Ht, h2_n

    def issue_mlp_head(c, Ht):
        """y matmuls + y_sb copy + G1 + relu z1 (fp8)."""
        t0 = c * L
        y_ps = z_ps.tile([128, 2, 512], f32, tag="z")
        nc.tensor.matmul(y_ps[:, 0, :], ytw[:, 0:128], Ht[:, 0, :], start=True, stop=False)
        nc.tensor.matmul(y_ps[:, 0, :], ytw[:, 128:256], Ht[:, 1, :], start=False, stop=False)
        nc.tensor.matmul(y_ps[:, 0, :], ytw[:, 256:384], xt[:, t0:t0 + 512],
                         start=False, stop=True)
        y_sb = ypool.tile([128, 512], f32r, tag="ysb")
        nc.scalar.copy(y_sb[:], y_ps[:, 0, :])

        z1 = zpool.tile([128, 4, 512], fp8, tag="z1")
        for half in range(2):
            zp = z_ps.tile([128, 2, 512], f32, tag="z")
            for mi in range(2):
                m = 2 * half + mi
                nc.tensor.matmul(zp[:, mi, :], g1[:, m * 128:(m + 1) * 128], y_sb[:],
                                 start=True, stop=True)
            if TUNE["relu_pair"]:
                nc.scalar.activation(z1[:, 2 * half:2 * half + 2, :], zp[:],
                                     ACT.Relu, bias=bias0[:])
            else:
                for mi in range(2):
                    m = 2 * half + mi
                    nc.scalar.activation(z1[:, m:m + 1, :], zp[:, mi, :],
                                         ACT.Relu, bias=bias0[:])
        return z1

    def _fp8_layer(gt, zin, zout_tag, width=512):
        """One fp8 DoubleRow 512->512 layer with relu -> new fp8 z tile."""
        zout = zpool.tile([128, 4, 512], fp8, tag=zout_tag)
        zps = []
        for half in range(2):
            zps.append(z_ps.tile([128, 2, 512], f32, tag="z", name=f"zp_{zout_tag}{half}"))
        if TUNE["g2_kp_interleave"]:
            order = [(half, mi, kp) for kp in range(2) for half in range(2)
                     for mi in range(2)]
        else:
            order = [(half, mi, kp) for half in range(2) for mi in range(2)
                     for kp in range(2)]
        for half, mi, kp in order:
            m = 2 * half + mi
            nc.tensor.matmul(zps[half][:, mi, :],
                             gt[:, 2 * kp:2 * kp + 2, m * 128:(m + 1) * 128],
                             zin[:, 2 * kp:2 * kp + 2, :],
                             start=(kp == 0), stop=(kp == 1), perf_mode=DR)
        for half in range(2):
            if TUNE["relu_pair"]:
                nc.scalar.activation(zout[:, 2 * half:2 * half + 2, :], zps[half][:],
                                     ACT.Relu, bias=bias0[:])
            else:
                for mi in range(2):
                    m = 2 * half + mi
                    nc.scalar.activation(zout[:, m:m + 1, :], zps[half][:, mi, :],
                                         ACT.Relu, bias=bias0[:])
        return zout

    def issue_mlp_tail(c, z1):
        t0 = c * L
        z2 = _fp8_layer(g2t, z1, "z2")
        z3 = _fp8_layer(g3t, z2, "z3")
        zp4 = z_ps.tile([128, 2, 512], f32, tag="z")
        for kp in range(2):
            nc.tensor.matmul(zp4[:, 0, :], g4t[:, 2 * kp:2 * kp + 2, :],
                             z3[:, 2 * kp:2 * kp + 2, :],
                             start=(kp == 0), stop=(kp == 1), perf_mode=DR)
        o_sb = opool.tile([128, 512], f32, tag="osb")
        nc.vector.tensor_tensor(o_sb[:], zp4[:, 0, :], xt[:, t0:t0 + 512].bitcast(f32),
                                AL.add)
        nc.sync.dma_start(out_d[:, t0:t0 + 512], o_sb[:])

    # ---- software-pipelined issue: slot c runs
    #      [bu_c | y,G1_{c-2} | tri_{c-1} | G2..G4_{c-2}] on the PE
    ups = {}
    Hts = {}
    z1s = {}
    for c in range(nchunk + 2):
        if c < nchunk:
            ups[c] = issue_bu(c)
        if c >= 2:
            z1s[c - 2] = issue_mlp_head(c - 2, Hts.pop(c - 2))
        if 1 <= c < nchunk + 1:
            Hts[c - 1], h2 = issue_scan(c - 1, ups.pop(c - 1), h2)
        if c >= 2:
            issue_mlp_tail(c - 2, z1s.pop(c - 2))


# ---------------------------------------------------------------- PJRT runner

def _make_runner(nc, n_cores):
    import jax
    from jax.sharding import Mesh, PartitionSpec
    from jax.experimental.shard_map import shard_map
    import concourse.mybir as mybir
    from concourse import bass2jax

    bass2jax.install_neuronx_cc_hook()
    assert nc.is_finalized()
    partition_name = nc.partition_id_tensor.name if nc.partition_id_tensor else None

    in_names, out_names, out_avals, zero_shapes = [], [], [], []
    for alloc in nc.m.functions[0].allocations:
        if not isinstance(alloc, mybir.MemoryLocationSet):
            continue
        name = alloc.memorylocations[0].name
        if alloc.kind == "ExternalInput":
            if name != partition_name:
                in_names.append(name)
        elif alloc.kind == "ExternalOutput":
            shape = tuple(alloc.tensor_shape)
            dtype = mybir.dt.np(alloc.dtype)
            out_names.append(name)
            out_avals.append(jax.core.ShapedArray(shape, dtype))
            zero_shapes.append((shape, dtype))
    n_params = len(in_names)
    n_outs = len(out_avals)
    all_in_names = list(in_names) + list(out_names)
    if partition_name is not None:
        all_in_names.append(partition_name)
    donate = tuple(range(n_params, n_params + n_outs))

    def _body(*args):
        operands = list(args)
        if partition_name is not None:
            operands.append(bass2jax.partition_id_tensor())
        outs = bass2jax._bass_exec_p.bind(
            *operands,
            out_avals=tuple(out_avals),
            in_names=tuple(all_in_names),
            out_names=tuple(out_names),
            lowering_input_output_aliases=(),
            sim_require_finite=True,
            sim_require_nnan=True,
            nc=nc,
        )
        return tuple(outs)

    devices = jax.devices()[:n_cores]
    if n_cores == 1:
        fn = jax.jit(_body, donate_argnums=donate, keep_unused=True)
    else:
        mesh = Mesh(np.asarray(devices), ("core",))
        fn = jax.jit(
            shard_map(_body, mesh=mesh,
                      in_specs=(PartitionSpec("core"),) * (n_params + n_outs),
                      out_specs=(PartitionSpec("core"),) * n_outs,
                      check_rep=False),
            donate_argnums=donate, keep_unused=True,
        )

    def run(per_core_inputs):
        if n_cores == 1:
            ins = [np.asarray(per_core_inputs[0][n]) for n in in_names]
            zeros = [np.zeros(s, d) for s, d in zero_shapes]
        else:
            ins = [np.concatenate([np.asarray(per_core_inputs[c][n])
                                   for c in range(n_cores)], axis=0) for n in in_names]
            zeros = [np.zeros((n_cores * s[0], *s[1:]), d) for s, d in zero_shapes]
        out_arrs = fn(*ins, *zeros)
        if n_cores == 1:
            return [{name: np.asarray(out_arrs[i]) for i, name in enumerate(out_names)}]
        res = []
        for c in range(n_cores):
            d = {}
            for i, name in enumerate(out_names):
                full = np.asarray(out_arrs[i])
                d[name] = full.reshape(n_cores, *out_avals[i].shape)[c]
            res.append(d)
        return res

    run.fn = fn
    run.in_names = in_names
    run.out_names = out_names
    run.zero_shapes = zero_shapes
    return run


_RUNNER = None


def _get_runner():
    global _RUNNER
    if _RUNNER is None:
        nc = _build_program(T)
        _RUNNER = _make_runner(nc, NCORES)
    return _RUNNER


def kernel(**inputs):
    import time as _time
    global _RUNNER
    p = {k: np.asarray(v) for k, v in inputs.items()}
    consts = _host_prep(p)
    x = p['x'].astype(np.float32)            # [B, T, D]
    per_core = []
    for b in range(B):
        m = dict(consts)
        m['xt'] = np.ascontiguousarray(x[b].T)
        per_core.append(m)
    res = None
    for attempt in range(3):
        try:
            run = _get_runner()
            res = run(per_core)
            break
        except Exception:
            # transient NRT exec faults have been observed on the first
            # execution of a freshly compiled NEFF; rebuild the jitted
            # callable (NEFF comes from the compile cache) and retry.
            _RUNNER = None
            if attempt == 2:
                raise
            _time.sleep(2.0)
    out = np.stack([res[b]['outT'].T for b in range(B)], axis=0)
    return np.ascontiguousarray(out, dtype=np.float32)


# revision 18
# speedup vs baseline: 1.5875x; 1.4393x over previous
"""Trainium2 Bass kernel for nn_DWNBlock (LRU scan + Lipschitz sandwich MLP).

Self-contained: host-side parameter folding (Cayley transforms, scan
constants) in numpy float64, then a fully-unrolled Tile/Bass SPMD program
on 8 NeuronCores, data-parallel over the batch dimension (one batch
element per core).

Device algorithm per core (x^T in channel-major [128, 8192]):
  1. Bu^(t-major) via PE matmuls (f32r), pre-scale by lam^-s (DVE, one
     merged broadcast op), complex combine into bf16 `up` (Pool)
  2. chunked linear-recurrence scan (L=512): shared upper-triangular-ones
     matmuls over in-chunk time (PE, bf16), sequential cross-chunk carry
     (tiny Pool/DVE column ops), post-scale by lam^t with the carry folded
     in (2 merged DVE ops + 1 Pool add)
  3. y^T = C_re@H_re - C_im@H_im + D@x^T (PE f32r, PSUM-accumulated)
  4. folded MLP in fp8e4m3 DoubleRow perf mode (2x PE rate):
     relu(G1 f32r) -> fp8; G2, G3 fp8-DR + relu->fp8; G4 fp8-DR;
     residual add + DMA out
Issue order is software-pipelined two chunks deep so the PE never waits
on the DVE/Pool pre-scale chain: per slot c the PE runs
[bu_c | y,G1_{c-2} | tri_{c-1} | G2..G4_{c-2}].
"""
import math
import os
import sys

for _p in ('/opt/trn_rl_repo',):
    if _p not in sys.path:
        sys.path.insert(0, _p)

import numpy as np
import ml_dtypes

D = 128          # d_model
N = 128          # d_state
H = 512          # MLP hidden
T = 8192         # sequence length
B = 8            # batch
L = 512          # scan chunk length
NCORES = 8
SCALE = 1.0
SQRT2 = math.sqrt(2.0)

E4M3 = ml_dtypes.float8_e4m3
BF16 = ml_dtypes.bfloat16

# schedule-tuning knobs
TUNE = dict(epool=2, upool=2, hpool=3, tpool=3, ypool=3, zpool=3, opool=3,
            bups=1, mps=2, zps=2,
            relu_pair=True,        # [128,1024] paired relus vs 4x[128,512]
            g2_kp_interleave=True)  # all-kp0 mms before kp1 mms in G2/G3


# ---------------------------------------------------------------- host prep

def _cayley64(W):
    cout, cin = W.shape
    if cin > cout:
        return _cayley64(W.T).T
    U, V = W[:cin], W[cin:]
    I = np.eye(cin, dtype=W.dtype)
    A = U - U.T + V.T @ V
    iIpA = np.linalg.inv(I + A)
    return np.concatenate([iIpA @ (I - A), -2.0 * V @ iIpA], axis=0)


def _host_prep(p):
    """Fold all parameters into device constants (float64 -> device dtypes)."""
    f8 = np.float64
    nu_log = p['nu_log'].astype(f8)
    theta_log = p['theta_log'].astype(f8)
    gamma_log = p['gamma_log'].astype(f8)
    lam = np.exp(-np.exp(nu_log)) * np.exp(1j * np.exp(theta_log))   # [N]
    Beff = np.exp(gamma_log)[:, None] * (p['B_re'].astype(f8) + 1j * p['B_im'].astype(f8))
    beff_w = np.concatenate([Beff.real.T, Beff.imag.T], axis=1)      # [D, 2N]

    C = p['C_re'].astype(f8) + 1j * p['C_im'].astype(f8)             # [D, N]
    ytw = np.concatenate([C.real.T, (-C.imag).T, p['Dmat'].astype(f8).T], axis=1)  # [128, 384]

    s = np.arange(L)
    loglam = np.log(lam)
    pneg = np.exp(-s[:, None] * loglam[None, :])    # [L, N] = lam^-s
    ppos = np.exp(s[None, :] * loglam[:, None])     # [N, L] = lam^t'
    lamL = lam ** L
    lamL1 = lam ** (L - 1)

    # pneg packed [128, 2048] = [variant0 (1024) | variant1 (1024)]
    #   variant0: [P0r|P0i|P1r|P1i|P2r|P2i|P3r|P3i]  (for e1 = bu * pneg)
    #   variant1: [P0i|P0r|P1i|P1r|P2i|P2r|P3i|P3r]  (for e2 = bu * pneg_sw)
    def _pk(j):   # s-tile j of pneg, [128, N]
        return pneg[j * 128:(j + 1) * 128, :]
    v0 = np.concatenate([a for j in range(4) for a in (_pk(j).real, _pk(j).imag)], axis=1)
    v1 = np.concatenate([a for j in range(4) for a in (_pk(j).imag, _pk(j).real)], axis=1)
    pneg_pack = np.concatenate([v0, v1], axis=1)                      # [128, 2048]

    # ppos packed [128, 2048] = [re | im | -im | re]
    # t13 = (m_re + a_re) * [re|im] ; t24 = (m_im + a_im) * [-im|re]
    # H = t13 + t24 = [Hre | Him]
    ppos_pack = np.concatenate([ppos.real, ppos.imag, -ppos.imag, ppos.real], axis=1)

    # tri_ones[s, u] = 1 if s <= u, [128, 512] (bf16 on device)
    tri_ones = (np.arange(128)[:, None] <= np.arange(512)[None, :]).astype(f8)

    # carry-chain per-partition scalar column pairs:
    # [lam_re|lam_im], [-lam_im|lam_re], [lamL_re|lamL_im], [-lamL_im|lamL_re],
    # [lamL1_re|lamL1_im], [-lamL1_im|lamL1_re]
    lamcols = np.stack([
        lam.real, lam.imag, -lam.imag, lam.real,
        lamL.real, lamL.imag, -lamL.imag, lamL.real,
        lamL1.real, lamL1.imag, -lamL1.imag, lamL1.real,
    ], axis=1)                                       # [128, 12]

    def _q(Wkey, akey, fout):
        Wd = p[Wkey].astype(f8)
        Q = _cayley64((float(p[akey][0]) / np.linalg.norm(Wd)) * Wd)
        return Q[:, fout:], Q[:, :fout]

    Q1in, Q1out = _q('W1', 'alpha1', H)
    Q2in, Q2out = _q('W2', 'alpha2', H)
    Q3in, Q3out = _q('W3', 'alpha3', H)
    Qlin = _cayley64((float(p['alphal'][0]) / np.linalg.norm(p['Wl'].astype(f8)))
                     * p['Wl'].astype(f8))[:, D:]    # [128, 512]

    e = np.exp
    ps1, ps2, ps3 = p['psi1'].astype(f8), p['psi2'].astype(f8), p['psi3'].astype(f8)
    G1 = SCALE * SCALE * SQRT2 * (Q1in.T * e(-ps1)[None, :])                    # [128, 512]
    G2 = 2.0 * SCALE * (e(ps1)[:, None] * Q1out) @ (Q2in.T * e(-ps2)[None, :])  # [512, 512]
    G3 = 2.0 * SCALE * (e(ps2)[:, None] * Q2out) @ (Q3in.T * e(-ps3)[None, :])  # [512, 512]
    G4 = SQRT2 * SCALE * (e(ps3)[:, None] * Q3out) @ Qlin.T                     # [512, 128]

    def _kpack(G, w):
        # [K, w] (K = 4*128) -> [128, 4*w] with [p, 512k + j] = G[128k + p, j]
        return np.concatenate([G[128 * k:128 * (k + 1), :] for k in range(4)], axis=1)

    out = {}
    for k, v in dict(beff_w=beff_w, ytw=ytw, pneg_pack=pneg_pack,
                     ppos_pack=ppos_pack, lamcols=lamcols, g1=G1).items():
        out[k] = np.ascontiguousarray(v, dtype=np.float32)
    out['tri_ones'] = np.ascontiguousarray(tri_ones, dtype=BF16)
    out['g2_8'] = np.ascontiguousarray(_kpack(G2, 512), dtype=np.float32).astype(E4M3)
    out['g3_8'] = np.ascontiguousarray(_kpack(G3, 512), dtype=np.float32).astype(E4M3)
    out['g4_8'] = np.ascontiguousarray(_kpack(G4, 128), dtype=np.float32).astype(E4M3)
    return out


# ---------------------------------------------------------------- device program

def _build_program(t_len, reps=1):
    from contextlib import nullcontext
    from concourse import bacc
    import concourse.mybir as mybir
    from concourse.tile import TileContext

    f32 = mybir.dt.float32
    f32r = mybir.dt.float32r
    bf16 = mybir.dt.bfloat16
    fp8 = mybir.dt.float8e4
    AL = mybir.AluOpType
    ACT = mybir.ActivationFunctionType
    DR = mybir.MatmulPerfMode.DoubleRow
    nchunk = t_len // L

    nc = bacc.Bacc("TRN2", target_bir_lowering=False, debug=False)

    xt_d = nc.dram_tensor("xt", [128, t_len], f32r, kind="ExternalInput").ap()
    beff_d = nc.dram_tensor("beff_w", [128, 256], f32r, kind="ExternalInput").ap()
    ytw_d = nc.dram_tensor("ytw", [128, 384], f32r, kind="ExternalInput").ap()
    pneg_d = nc.dram_tensor("pneg_pack", [128, 2048], f32, kind="ExternalInput").ap()
    ppos_d = nc.dram_tensor("ppos_pack", [128, 2048], f32, kind="ExternalInput").ap()
    tri_d = nc.dram_tensor("tri_ones", [128, 512], bf16, kind="ExternalInput").ap()
    lamc_d = nc.dram_tensor("lamcols", [128, 12], f32, kind="ExternalInput").ap()
    g1_d = nc.dram_tensor("g1", [128, 512], f32r, kind="ExternalInput").ap()
    g2_d = nc.dram_tensor("g2_8", [128, 2048], fp8, kind="ExternalInput").ap()
    g3_d = nc.dram_tensor("g3_8", [128, 2048], fp8, kind="ExternalInput").ap()
    g4_d = nc.dram_tensor("g4_8", [128, 512], fp8, kind="ExternalInput").ap()
    out_d = nc.dram_tensor("outT", [128, t_len], f32, kind="ExternalOutput").ap()

    with TileContext(nc) as tc:
        with (
            tc.tile_pool(name="const", bufs=1) as cpool,
            tc.tile_pool(name="epool", bufs=TUNE["epool"]) as epool,
            tc.tile_pool(name="upool", bufs=TUNE["upool"]) as upool,
            tc.tile_pool(name="hpool", bufs=TUNE["hpool"]) as hpool,
            tc.tile_pool(name="tpool", bufs=TUNE["tpool"]) as tpool,
            tc.tile_pool(name="ypool", bufs=TUNE["ypool"]) as ypool,
            tc.tile_pool(name="zpool", bufs=TUNE["zpool"]) as zpool,
            tc.tile_pool(name="opool", bufs=TUNE["opool"]) as opool,
            tc.tile_pool(name="carry", bufs=3) as carry_pool,
            tc.tile_pool(name="bups", bufs=TUNE["bups"], space="PSUM") as bu_ps,
            tc.tile_pool(name="mps", bufs=TUNE["mps"], space="PSUM") as m_ps,
            tc.tile_pool(name="zps", bufs=TUNE["zps"], space="PSUM") as z_ps,
        ):
            # ---- constants into SBUF
            xt = cpool.tile([128, t_len], f32r, tag="xt")
            for q in range(max(1, t_len // 2048)):
                w = min(2048, t_len)
                nc.sync.dma_start(xt[:, q * w:(q + 1) * w], xt_d[:, q * w:(q + 1) * w])
            beff = cpool.tile([128, 256], f32r, tag="beff")
            nc.sync.dma_start(beff[:], beff_d[:])
            ytw = cpool.tile([128, 384], f32r, tag="ytw")
            nc.sync.dma_start(ytw[:], ytw_d[:])
            pneg3 = cpool.tile([128, 2, 1024], f32, tag="pneg")
            nc.sync.dma_start(pneg3[:], pneg_d[:])
            ppos3 = cpool.tile([128, 4, 512], f32, tag="ppos")
            nc.sync.dma_start(ppos3[:], ppos_d[:])
            tri = cpool.tile([128, 512], bf16, tag="tri")
            nc.sync.dma_start(tri[:], tri_d[:])
            lamc = cpool.tile([128, 12], f32, tag="lamc")
            nc.sync.dma_start(lamc[:], lamc_d[:])
            g1 = cpool.tile([128, 512], f32r, tag="g1")
            nc.sync.dma_start(g1[:], g1_d[:])
            g2t = cpool.tile([128, 4, 512], fp8, tag="g2")
            nc.sync.dma_start(g2t[:], g2_d[:])
            g3t = cpool.tile([128, 4, 512], fp8, tag="g3")
            nc.sync.dma_start(g3t[:], g3_d[:])
            g4t = cpool.tile([128, 4, 128], fp8, tag="g4")
            nc.sync.dma_start(g4t[:], g4_d[:])
            bias0 = cpool.tile([128, 1], f32, tag="bias0")
            nc.vector.memset(bias0[:], 0.0)

            env = dict(nc=nc, mybir=mybir, nchunk=nchunk, xt=xt, beff=beff,
                       ytw=ytw, pneg3=pneg3, ppos3=ppos3, tri=tri, lamc=lamc,
                       g1=g1, g2t=g2t, g3t=g3t, g4t=g4t, bias0=bias0,
                       out_d=out_d, epool=epool, upool=upool, hpool=hpool,
                       tpool=tpool, ypool=ypool, zpool=zpool, opool=opool,
                       carry_pool=carry_pool, bu_ps=bu_ps, m_ps=m_ps, z_ps=z_ps)

            loop_cm = tc.For_i(0, reps) if reps > 1 else nullcontext()
            with loop_cm:
                _chunk_loop_body(env)

    nc.finalize()
    return nc


def _chunk_loop_body(env):
    nc = env['nc']
    mybir = env['mybir']
    f32 = mybir.dt.float32
    f32r = mybir.dt.float32r
    bf16 = mybir.dt.bfloat16
    fp8 = mybir.dt.float8e4
    AL = mybir.AluOpType
    ACT = mybir.ActivationFunctionType
    DR = mybir.MatmulPerfMode.DoubleRow

    nchunk = env['nchunk']
    xt, beff, ytw = env['xt'], env['beff'], env['ytw']
    pneg3, ppos3, tri, lamc = env['pneg3'], env['ppos3'], env['tri'], env['lamc']
    g1, g2t, g3t, g4t, bias0 = env['g1'], env['g2t'], env['g3t'], env['g4t'], env['bias0']
    out_d = env['out_d']
    epool, upool, hpool, tpool = env['epool'], env['upool'], env['hpool'], env['tpool']
    ypool, zpool, opool, carry_pool = env['ypool'], env['zpool'], env['opool'], env['carry_pool']
    bu_ps, m_ps, z_ps = env['bu_ps'], env['m_ps'], env['z_ps']

    # lamcols column-pair views: [x_re|x_im] and [-x_im|x_re]
    lam_p, lam_q = lamc[:, 0:2], lamc[:, 2:4]
    lamL_p, lamL_q = lamc[:, 4:6], lamc[:, 6:8]
    lamL1_p, lamL1_q = lamc[:, 8:10], lamc[:, 10:12]

    # carry state h_0 = 0   ([128, 2] = [re|im])
    h2 = carry_pool.tile([128, 2], f32, tag="h2")
    nc.vector.memset(h2[:], 0.0)

    def issue_bu(c):
        """Bu matmuls + lam^-s pre-scale + complex combine -> up (bf16)."""
        t0 = c * L
        bu = bu_ps.tile([128, 1024], f32, tag="bu")
        for i in range(4):
            lhs = xt[:, t0 + i * 128: t0 + (i + 1) * 128]
            nc.tensor.matmul(bu[:, i * 256:(i + 1) * 256], lhs, beff[:],
                             start=True, stop=True)
        e12 = epool.tile([128, 2, 4, 256], f32, tag="e12")
        nc.vector.tensor_tensor(e12[:], bu[:, None, :].broadcast_to([128, 2, 1024]),
                                pneg3[:], AL.mult)
        up = upool.tile([128, 4, 256], bf16, tag="up")
        nc.gpsimd.tensor_tensor(up[:, :, 0:128], e12[:, 0, :, 0:128],
                                e12[:, 0, :, 128:256], AL.subtract)
        nc.gpsimd.tensor_tensor(up[:, :, 128:256], e12[:, 1, :, 0:128],
                                e12[:, 1, :, 128:256], AL.add)
        return up

    def issue_scan(c, up, h2):
        """tri matmuls, carry chain, post-scale -> H [128, 2, 512] f32r."""
        m_re = m_ps.tile([128, 512], f32, tag="m")
        m_im = m_ps.tile([128, 512], f32, tag="m")
        for j in range(4):
            nc.tensor.matmul(m_re[:, 128 * j:512], up[:, j, 0:128],
                             tri[:, 0:512 - 128 * j], start=(j == 0), stop=(j == 3))
        for j in range(4):
            nc.tensor.matmul(m_im[:, 128 * j:512], up[:, j, 128:256],
                             tri[:, 0:512 - 128 * j], start=(j == 0), stop=(j == 3))

        # a = lam * h  (Pool, SBUF-only; [128,2] complex as [re|im])
        ppa = carry_pool.tile([128, 2], f32, tag="ppa")
        ppb = carry_pool.tile([128, 2], f32, tag="ppb")
        a2 = carry_pool.tile([128, 2], f32, tag="a2")
        nc.gpsimd.tensor_tensor(ppa[:], h2[:, 0:1].broadcast_to([128, 2]), lam_p, AL.mult)
        nc.gpsimd.tensor_tensor(ppb[:], h2[:, 1:2].broadcast_to([128, 2]), lam_q, AL.mult)
        nc.gpsimd.tensor_tensor(a2[:], ppa[:], ppb[:], AL.add)

        # mp = lamL1 * M[:, L-1]   (DVE: reads PSUM column)
        mr_col = m_re[:, L - 1:L]
        mi_col = m_im[:, L - 1:L]
        ppe = carry_pool.tile([128, 2], f32, tag="ppe")
        ppf = carry_pool.tile([128, 2], f32, tag="ppf")
        mp2 = carry_pool.tile([128, 2], f32, tag="mp2")
        nc.vector.tensor_tensor(ppe[:], mr_col.broadcast_to([128, 2]), lamL1_p, AL.mult)
        nc.vector.tensor_tensor(ppf[:], mi_col.broadcast_to([128, 2]), lamL1_q, AL.mult)
        nc.vector.tensor_tensor(mp2[:], ppe[:], ppf[:], AL.add)

        # h' = lamL * h + mp  (Pool)
        ppc = carry_pool.tile([128, 2], f32, tag="ppc")
        ppd = carry_pool.tile([128, 2], f32, tag="ppd")
        s2 = carry_pool.tile([128, 2], f32, tag="s2")
        h2_n = carry_pool.tile([128, 2], f32, tag="h2")
        nc.gpsimd.tensor_tensor(ppc[:], h2[:, 0:1].broadcast_to([128, 2]), lamL_p, AL.mult)
        nc.gpsimd.tensor_tensor(ppd[:], h2[:, 1:2].broadcast_to([128, 2]), lamL_q, AL.mult)
        nc.gpsimd.tensor_tensor(s2[:], ppc[:], ppd[:], AL.add)
        nc.gpsimd.tensor_tensor(h2_n[:], s2[:], mp2[:], AL.add)

        # post-scale: t13 = (m_re + a_re) * [re|im], t24 = (m_im + a_im) * [-im|re]
        t13 = tpool.tile([128, 2, 512], f32, tag="t13")
        t24 = tpool.tile([128, 2, 512], f32, tag="t24")
        nc.vector.scalar_tensor_tensor(t13[:], m_re[:, None, :].broadcast_to([128, 2, 512]),
                                       a2[:, 0:1], ppos3[:, 0:2, :], AL.add, AL.mult)
        nc.vector.scalar_tensor_tensor(t24[:], m_im[:, None, :].broadcast_to([128, 2, 512]),
                                       a2[:, 1:2], ppos3[:, 2:4, :], AL.add, AL.mult)
        Ht = hpool.tile([128, 2, 512], f32r, tag="H")
        nc.gpsimd.tensor_tensor(Ht[:], t13[:], t24[:], AL.add)
        return Ht, h2_n

    def issue_y(c, Ht, y_ps):
        """y matmuls + y_sb copy (Act). y_ps is half of a shared y/z4 tile."""
        t0 = c * L
        nc.tensor.matmul(y_ps[:], ytw[:, 0:128], Ht[:, 0, :], start=True, stop=False)
        nc.tensor.matmul(y_ps[:], ytw[:, 128:256], Ht[:, 1, :], start=False, stop=False)
        nc.tensor.matmul(y_ps[:], ytw[:, 256:384], xt[:, t0:t0 + 512],
                         start=False, stop=True)
        y_sb = ypool.tile([128, 512], f32r, tag="ysb")
        nc.scalar.copy(y_sb[:], y_ps[:])
        return y_sb

    def issue_g1relu(c, y_sb):
        """G1 (f32r) + relu z1 (fp8)."""
        z1 = zpool.tile([128, 4, 512], fp8, tag="z1")
        for half in range(2):
            zp = z_ps.tile([128, 2, 512], f32, tag="z", name=f"zp_g1_{half}")
            for mi in range(2):
                m = 2 * half + mi
                nc.tensor.matmul(zp[:, mi, :], g1[:, m * 128:(m + 1) * 128], y_sb[:],
                                 start=True, stop=True)
            nc.scalar.activation(z1[:, 2 * half:2 * half + 2, :], zp[:],
                                 ACT.Relu, bias=bias0[:])
        return z1

    def _fp8_layer(gt, zin, zout_tag):
        """One fp8 DoubleRow 512->512 layer with relu -> new fp8 z tile."""
        zout = zpool.tile([128, 4, 512], fp8, tag=zout_tag, name=f"zt_{zout_tag}")
        zps = []
        for half in range(2):
            zps.append(z_ps.tile([128, 2, 512], f32, tag="z", name=f"zp_{zout_tag}{half}"))
        if TUNE["g2_kp_interleave"]:
            order = [(half, mi, kp) for kp in range(2) for half in range(2)
                     for mi in range(2)]
        else:
            order = [(half, mi, kp) for half in range(2) for mi in range(2)
                     for kp in range(2)]
        for half, mi, kp in order:
            m = 2 * half + mi
            nc.tensor.matmul(zps[half][:, mi, :],
                             gt[:, 2 * kp:2 * kp + 2, m * 128:(m + 1) * 128],
                             zin[:, 2 * kp:2 * kp + 2, :],
                             start=(kp == 0), stop=(kp == 1), perf_mode=DR)
        for half in range(2):
            nc.scalar.activation(zout[:, 2 * half:2 * half + 2, :], zps[half][:],
                                 ACT.Relu, bias=bias0[:])
        return zout

    def issue_g4(c, z3, zp4):
        """G4 matmuls into half of the shared y/z4 tile."""
        for kp in range(2):
            nc.tensor.matmul(zp4[:], g4t[:, 2 * kp:2 * kp + 2, :],
                             z3[:, 2 * kp:2 * kp + 2, :],
                             start=(kp == 0), stop=(kp == 1), perf_mode=DR)
        return zp4

    def issue_osb(c, zp4):
        t0 = c * L
        o_sb = opool.tile([128, 512], f32, tag="osb")
        nc.vector.tensor_tensor(o_sb[:], zp4[:], xt[:, t0:t0 + 512].bitcast(f32),
                                AL.add)
        nc.sync.dma_start(out_d[:, t0:t0 + 512], o_sb[:])

    # ---- 5-deep software-pipelined issue. Per-slot stage order is a tuning
    # knob (TUNE['order']): each entry is (stage, lag); stage X at lag k
    # processes chunk c-k in slot c. Every PE op's cross-engine inputs are
    # produced in an earlier slot, so relu/post-scale latency is hidden by
    # other chunks' matmuls.
    ups, Hts, ysbs, z1s, z2s, z3s, zp4s = {}, {}, {}, {}, {}, {}, {}
    order = TUNE.get('order', [('bu', 0), ('g1', 3), ('yz4', None), ('scan', 1),
                               ('g2', 3), ('g3', 4), ('osb', 5)])
    Y_LAG, G4_LAG = TUNE.get('y_lag', 2), TUNE.get('g4_lag', 5)
    lags = [lag for _, lag in order if lag is not None] + [Y_LAG, G4_LAG]
    maxlag = max(lags)
    for c in range(nchunk + maxlag):
        for stage, lag in order:
            if stage == 'yz4':
                # shared [128,2,512] psum tile: half 0 = y_{c-Y_LAG},
                # half 1 = z4_{c-G4_LAG}; allocated once per slot.
                iy, i4 = c - Y_LAG, c - G4_LAG
                do_y, do_4 = 0 <= iy < nchunk, 0 <= i4 < nchunk
                if not (do_y or do_4):
                    continue
                yz = z_ps.tile([128, 2, 512], f32, tag="z", name="yz")
                if do_4:
                    zp4s[i4] = issue_g4(i4, z3s.pop(i4), yz[:, 1, :])
                if do_y:
                    ysbs[iy] = issue_y(iy, Hts.pop(iy), yz[:, 0, :])
                continue
            i = c - lag
            if not (0 <= i < nchunk):
                continue
            if stage == 'bu':
                ups[i] = issue_bu(i)
            elif stage == 'g1':
                z1s[i] = issue_g1relu(i, ysbs.pop(i))
            elif stage == 'scan':
                Hts[i], h2 = issue_scan(i, ups.pop(i), h2)
            elif stage == 'g2':
                z2s[i] = _fp8_layer(g2t, z1s.pop(i), "z2")
            elif stage == 'g3':
                z3s[i] = _fp8_layer(g3t, z2s.pop(i), "z3")
            elif stage == 'osb':
                issue_osb(i, zp4s.pop(i))


# ---------------------------------------------------------------- PJRT runner

def _make_runner(nc, n_cores):
    import jax
    from jax.sharding import Mesh, PartitionSpec
    from jax.experimental.shard_map import shard_map
    import concourse.mybir as mybir
    from concourse import bass2jax

    bass2jax.install_neuronx_cc_hook()
    assert nc.is_finalized()
    partition_name = nc.partition_id_tensor.name if nc.partition_id_tensor else None

    in_names, out_names, out_avals, zero_shapes = [], [], [], []
    for alloc in nc.m.functions[0].allocations:
        if not isinstance(alloc, mybir.MemoryLocationSet):
            continue
        name = alloc.memorylocations[0].name
        if alloc.kind == "ExternalInput":
            if name != partition_name:
                in_names.append(name)
        elif alloc.kind == "ExternalOutput":
            shape = tuple(alloc.tensor_shape)
            dtype = mybir.dt.np(alloc.dtype)
            out_names.append(name)
            out_avals.append(jax.core.ShapedArray(shape, dtype))
            zero_shapes.append((shape, dtype))
    n_params = len(in_names)
    n_outs = len(out_avals)
    all_in_names = list(in_names) + list(out_names)
    if partition_name is not None:
        all_in_names.append(partition_name)
    donate = tuple(range(n_params, n_params + n_outs))

    def _body(*args):
        operands = list(args)
        if partition_name is not None:
            operands.append(bass2jax.partition_id_tensor())
        outs = bass2jax._bass_exec_p.bind(
            *operands,
            out_avals=tuple(out_avals),
            in_names=tuple(all_in_names),
            out_names=tuple(out_names),
            lowering_input_output_aliases=(),
            sim_require_finite=True,
            sim_require_nnan=True,
            nc=nc,
        )
        return tuple(outs)

    devices = jax.devices()[:n_cores]
    if n_cores == 1:
        fn = jax.jit(_body, donate_argnums=donate, keep_unused=True)
    else:
        mesh = Mesh(np.asarray(devices), ("core",))
        fn = jax.jit(
            shard_map(_body, mesh=mesh,
                      in_specs=(PartitionSpec("core"),) * (n_params + n_outs),
                      out_specs=(PartitionSpec("core"),) * n_outs,
                      check_rep=False),
            donate_argnums=donate, keep_unused=True,
        )

    def run(per_core_inputs):
        if n_cores == 1:
            ins = [np.asarray(per_core_inputs[0][n]) for n in in_names]
            zeros = [np.zeros(s, d) for s, d in zero_shapes]
        else:
            ins = [np.concatenate([np.asarray(per_core_inputs[c][n])
                                   for c in range(n_cores)], axis=0) for n in in_names]
            zeros = [np.zeros((n_cores * s[0], *s[1:]), d) for s, d in zero_shapes]
        out_arrs = fn(*ins, *zeros)
        if n_cores == 1:
            return [{name: np.asarray(out_arrs[i]) for i, name in enumerate(out_names)}]
        res = []
        for c in range(n_cores):
            d = {}
            for i, name in enumerate(out_names):
                full = np.asarray(out_arrs[i])
                d[name] = full.reshape(n_cores, *out_avals[i].shape)[c]
            res.append(d)
        return res

    run.fn = fn
    run.in_names = in_names
    run.out_names = out_names
    run.zero_shapes = zero_shapes
    return run


_RUNNER = None


def _get_runner():
    global _RUNNER
    if _RUNNER is None:
        nc = _build_program(T)
        _RUNNER = _make_runner(nc, NCORES)
    return _RUNNER


def kernel(**inputs):
    import time as _time
    global _RUNNER
    p = {k: np.asarray(v) for k, v in inputs.items()}
    consts = _host_prep(p)
    x = p['x'].astype(np.float32)            # [B, T, D]
    per_core = []
    for b in range(B):
        m = dict(consts)
        m['xt'] = np.ascontiguousarray(x[b].T)
        per_core.append(m)
    res = None
    for attempt in range(3):
        try:
            run = _get_runner()
            res = run(per_core)
            break
        except Exception:
            # transient NRT exec faults have been observed on the first
            # execution of a freshly compiled NEFF; rebuild the jitted
            # callable (NEFF comes from the compile cache) and retry.
            _RUNNER = None
            if attempt == 2:
                raise
            _time.sleep(2.0)
    out = np.stack([res[b]['outT'].T for b in range(B)], axis=0)
    return np.ascontiguousarray(out, dtype=np.float32)
